# revision 23
# baseline (speedup 1.0000x reference)
"""CIF (continuous integrate-and-fire) segment-reduce kernel for 8 trn2 NeuronCores.

Math: with alphas rescaled per batch so sum_t alpha = target_len, the scan
    integrate += alpha; fire when integrate >= 0.95; subtract 1 at fire
has the closed form  n_t = floor(csum_t + 0.05)  (number of fires through t),
valid because every alpha < 1.  Each step t contributes to at most two output
tokens:
    c1_t = alpha_t - c2_t           -> token n_{t-1}
    c2_t = fire_t * (csum_t - n_t)  -> token n_t        (fire_t = n_t - n_{t-1})
Contributions to tokens >= target_len are dropped (trailing partial frame).
So   out[b] = C_b[L, T] @ hidden[b][T, H]   with a banded C built on-device
from the cumsum, and the matmul done on the PE in fp16 (1 cycle/row; fp32r
runs at ~2) with fp32 PSUM accumulation.

The cumsum itself is computed on the host in float64 (64K adds — descriptor-
style preprocessing; fp32 accumulation at magnitude ~256 would jitter fire
positions by ~1e-4 and flip occasional fire steps vs the reference).

Sharding: pure data parallel, 4 batches per core.
"""

import sys

if "/opt/trn_rl_repo" not in sys.path:
    sys.path.insert(0, "/opt/trn_rl_repo")

import numpy as np

import concourse.bass as bass
import concourse.bacc as bacc
import concourse.tile as tile
from concourse import mybir
from concourse import bass_utils

F32 = mybir.dt.float32
F32R = mybir.dt.float32r
F16 = mybir.dt.float16
I32 = mybir.dt.int32
A = mybir.AluOpType

# Problem constants (hardcoded per the harness contract).
B, T, H, L = 32, 2000, 512, 256
NCORES = 8
BL = B // NCORES  # 4 batches per core
THRESH_OFF = 1.0 - 0.95  # fire threshold offset: n_t = floor(csum + 0.05)


def build_nc(bl=BL, t=T, h=H, l=L, n_cores=NCORES):
    """Build the bass program for one core (SPMD across n_cores)."""
    tp = ((t + 127) // 128) * 128  # padded time
    nt = tp // 128                 # number of T-tiles
    ncol = bl * nt                 # columns in the [128, ncol] "column layout"

    nc = bacc.Bacc(
        "TRN2",
        target_bir_lowering=False,
        debug=False,
        num_devices=n_cores,
    )

    # h is host-interleaved to [t, bl*h]: row t holds all bl batches' step-t
    # vectors back to back -> fat contiguous DMA packets per SBUF partition.
    # fp16: halves DMA bytes and runs the PE at 1 cycle/row (fp32r is ~2).
    h_d = nc.dram_tensor("h", [t, bl * h], F16, kind="ExternalInput").ap()
    # csumz[b, 0] = 0; csumz[b, 1+j] = cumsum(rescaled alphas)[j] (flat pad)
    cz_d = nc.dram_tensor("csumz", [bl, tp + 1], F32, kind="ExternalInput").ap()
    tgtbc_d = nc.dram_tensor("tgt_bc", [128, ncol], F32, kind="ExternalInput").ap()
    iota_d = nc.dram_tensor("iota", [128, l], F16, kind="ExternalInput").ap()
    ident_d = nc.dram_tensor("ident", [bl, bl], F32, kind="ExternalInput").ap()
    # out is interleaved the same way: [l, bl*h]; host de-interleaves.
    out_d = nc.dram_tensor("out", [l, bl * h], F32, kind="ExternalOutput").ap()

    with tile.TileContext(nc) as tc:
        _body(tc, nc, h_d, cz_d, tgtbc_d, iota_d, ident_d, out_d,
              bl=bl, t=t, h=h, l=l, tp=tp, nt=nt, ncol=ncol)

    nc.compile()
    return nc


def _body(tc, nc, h_d, cz_d, tgtbc_d, iota_d, ident_d, out_d,
          *, bl, t, h, l, tp, nt, ncol):
    with (
        tc.tile_pool(name="const", bufs=1) as cpool,
        tc.tile_pool(name="prep", bufs=1) as prep,
        tc.tile_pool(name="pcol", bufs=1) as pcol,
    ):
        # ---- constants from host ----
        iota_sb = cpool.tile([128, l], F16)
        nc.sync.dma_start(iota_sb[:], iota_d[:])
        ident_sb = cpool.tile([bl, bl], F32)
        nc.sync.dma_start(ident_sb[:], ident_d[:])
        tgtbc_sb = cpool.tile([128, ncol], F32)
        nc.sync.dma_start(tgtbc_sb[:], tgtbc_d[:])

        cz_row = prep.tile([bl, tp + 1], F32)
        nc.sync.dma_start(cz_row[:], cz_d[:])

        # ---- transpose csum / csum_prev into column layout [128, ncol] ----
        # column (i*bl + b) of cs_all = csum[b, 128*i + p] = cz_row[b, 1+128i+p]
        cs_all = pcol.tile([128, ncol], F32)
        csp_all = pcol.tile([128, ncol], F32)
        with tc.tile_pool(name="tpp", bufs=4, space="PSUM") as tpp:
            for i in range(nt):
                pt = tpp.tile([128, bl], F32, tag="pt")
                nc.tensor.transpose(
                    pt[:], cz_row[0:bl, 1 + i * 128:1 + (i + 1) * 128],
                    ident_sb[:])
                nc.vector.tensor_copy(cs_all[:, i * bl:(i + 1) * bl], pt[:])
                pt2 = tpp.tile([128, bl], F32, tag="pt")
                nc.tensor.transpose(
                    pt2[:], cz_row[0:bl, i * 128:(i + 1) * 128], ident_sb[:])
                nc.vector.tensor_copy(csp_all[:, i * bl:(i + 1) * bl], pt2[:])

        # ---- column-layout math: coefficients ----
        def floor_col(dst, src):
            # dst = floor(src + THRESH_OFF), exact for 0 <= src < 2^23
            x = pcol.tile([128, ncol], F32, tag="fl_x")
            xi = pcol.tile([128, ncol], I32, tag="fl_i")
            xf = pcol.tile([128, ncol], F32, tag="fl_f")
            nc.vector.tensor_scalar(x[:], src[:], THRESH_OFF, None, A.add)
            nc.vector.tensor_copy(xi[:], x[:])   # f32 -> i32 (round or trunc)
            nc.vector.tensor_copy(xf[:], xi[:])  # i32 -> f32 (exact)
            nc.vector.tensor_tensor(dst[:], xf[:], x[:], A.is_gt)
            nc.vector.tensor_tensor(dst[:], xf[:], dst[:], A.subtract)

        al_all = pcol.tile([128, ncol], F32)
        nc.vector.tensor_tensor(al_all[:], cs_all[:], csp_all[:], A.subtract)

        n_all = pcol.tile([128, ncol], F32)
        floor_col(n_all, cs_all)
        np_all = pcol.tile([128, ncol], F32)
        floor_col(np_all, csp_all)

        fire = pcol.tile([128, ncol], F32)
        nc.vector.tensor_tensor(fire[:], n_all[:], np_all[:], A.subtract)
        d_all = pcol.tile([128, ncol], F32)
        nc.vector.tensor_tensor(d_all[:], cs_all[:], n_all[:], A.subtract)
        c2 = pcol.tile([128, ncol], F32)
        nc.vector.tensor_tensor(c2[:], fire[:], d_all[:], A.mult)
        c1 = pcol.tile([128, ncol], F32)
        nc.vector.tensor_tensor(c1[:], al_all[:], c2[:], A.subtract)

        # validity: contributions to token >= target are dropped
        v_t = pcol.tile([128, ncol], F32, tag="v_t")
        nc.vector.tensor_tensor(v_t[:], np_all[:], tgtbc_sb[:], A.is_lt)
        nc.vector.tensor_tensor(c1[:], c1[:], v_t[:], A.mult)
        v_t2 = pcol.tile([128, ncol], F32, tag="v_t2")
        nc.vector.tensor_tensor(v_t2[:], n_all[:], tgtbc_sb[:], A.is_lt)
        nc.vector.tensor_tensor(c2[:], c2[:], v_t2[:], A.mult)

        # ---- PE warm-up: ~7us of dummy matmuls gated on c1 so they run
        # right before the real matmul stream (HAM clock gate at 2.4 GHz) ----
        with tc.tile_pool(name="warm", bufs=1, space="PSUM") as wp:
            wps = wp.tile([128, l], F32)
            wm = min(ncol, 128)
            wn = min(l, ncol)
            for _ in range(8):
                nc.tensor.matmul(
                    wps[0:wm, 0:wn], c1[:, 0:wm], cs_all[:, 0:wn],
                    start=True, stop=True)

        # ---- main loop: build C^T tiles, matmul, accumulate ----
        nl = l // 128  # number of 128-row output chunks (2)
        with (
            tc.tile_pool(name="hin", bufs=6) as hp,
            tc.tile_pool(name="ct", bufs=16) as ctp,
            tc.tile_pool(name="acc", bufs=1, space="PSUM") as accp,
            tc.tile_pool(name="osb", bufs=4) as osb,
        ):
            psums = [
                [accp.tile([128, h], F32, tag=f"ps{b}{c}", name=f"ps{b}{c}")
                 for c in range(nl)]
                for b in range(bl)
            ]
            for i in range(nt):
                h_t = hp.tile([128, bl * h], F16, tag="h_t")
                tlo, thi = i * 128, min(t, (i + 1) * 128)
                if thi - tlo < 128:
                    nc.gpsimd.memset(h_t[64:128, :], 0.0)
                nc.sync.dma_start(h_t[0:thi - tlo, :], h_d[tlo:thi, :])

                for b in range(bl):
                    col = i * bl + b
                    # two dual-op compares on DVE (c1 at tok1, c2 at
                    # tok2 = n), merged by a plain add on GpSimd (the only
                    # TensorScalarPtr-free op Pool supports); deep pools
                    # below decouple the two engines.
                    ct = ctp.tile([128, l], F16, tag="ct")
                    tsa = ctp.tile([128, l], F16, tag="tsa")
                    tsb = ctp.tile([128, l], F16, tag="tsb")
                    nc.vector.tensor_scalar(
                        tsa[:], iota_sb[:], np_all[:, col:col + 1],
                        c1[:, col:col + 1], A.is_equal, A.mult)
                    nc.vector.tensor_scalar(
                        tsb[:], iota_sb[:], n_all[:, col:col + 1],
                        c2[:, col:col + 1], A.is_equal, A.mult)
                    nc.gpsimd.tensor_tensor(ct[:], tsa[:], tsb[:], A.add)

                    for c in range(nl):
                        nc.tensor.matmul(
                            psums[b][c][:],
                            ct[:, c * 128:(c + 1) * 128],
                            h_t[:, b * h:(b + 1) * h],
                            start=(i == 0), stop=(i == nt - 1))

            for c in range(nl):
                ot = osb.tile([128, bl * h], F32, tag="ot")
                for b in range(bl):
                    # split the tail evacuations across ACT and DVE
                    eng = nc.scalar.copy if b % 2 == 0 else nc.vector.tensor_copy
                    eng(ot[:, b * h:(b + 1) * h], psums[b][c][:])
                nc.sync.dma_start(out_d[c * 128:(c + 1) * 128, :], ot[:])


_cached_nc = None


def _get_nc():
    global _cached_nc
    if _cached_nc is None:
        _cached_nc = build_nc()
    return _cached_nc


def make_in_maps(hidden, alphas, target_lengths, bl=BL, t=T, l=L, n_cores=NCORES):
    tp = ((t + 127) // 128) * 128
    nt = tp // 128
    hidden = np.ascontiguousarray(np.asarray(hidden, dtype=np.float32))
    tl = np.asarray(target_lengths)

    # host-side f64 cumsum of rescaled alphas (exact-arithmetic fire positions)
    a64 = np.asarray(alphas, dtype=np.float64)
    scale = tl.astype(np.float64) / a64.sum(axis=1)
    csum = np.cumsum(a64 * scale[:, None], axis=1)  # [B, t]
    csumz = np.zeros((a64.shape[0], tp + 1), np.float64)
    csumz[:, 1:1 + t] = csum
    csumz[:, 1 + t:] = csum[:, -1:]
    csumz = csumz.astype(np.float32)

    iota = np.ascontiguousarray(
        np.broadcast_to(np.arange(l, dtype=np.float16)[None, :], (128, l)))
    ident = np.eye(bl, dtype=np.float32)
    in_maps = []
    for c in range(n_cores):
        sl = slice(c * bl, (c + 1) * bl)
        tgt = tl[sl].astype(np.float32)
        # interleave batches along the feature axis: h_r[t, b*h:(b+1)*h]
        h_r = np.ascontiguousarray(
            hidden[sl].transpose(1, 0, 2).reshape(t, bl * hidden.shape[2])
            .astype(np.float16))
        in_maps.append({
            "h": h_r,
            "csumz": np.ascontiguousarray(csumz[sl]),
            "tgt_bc": np.ascontiguousarray(
                np.broadcast_to(np.tile(tgt, nt)[None, :], (128, bl * nt))),
            "iota": iota,
            "ident": ident,
        })
    return in_maps


def kernel(hidden, alphas, target_lengths):
    nc = _get_nc()
    in_maps = make_in_maps(hidden, alphas, target_lengths)
    res = bass_utils.run_bass_kernel_spmd(
        nc, in_maps, core_ids=list(range(NCORES)))
    return assemble_out(res.results)


def assemble_out(results):
    # de-interleave [l, bl*h] -> [bl, l, h] per core
    outs = [r["out"].reshape(L, BL, H).transpose(1, 0, 2) for r in results]
    return np.ascontiguousarray(np.concatenate(outs, axis=0))


if __name__ == "__main__":
    # smoke test with random inputs
    rng = np.random.default_rng(0)
    hidden = rng.standard_normal((B, T, H), dtype=np.float32)
    alphas = rng.random((B, T), dtype=np.float32)
    tl = rng.integers(64, L + 1, size=(B,)).astype(np.int64)
    out = kernel(hidden, alphas, tl)
    print("out", out.shape, out.dtype, float(np.abs(out).sum()))


# revision 24
# speedup vs baseline: 1.1027x; 1.1027x over previous
"""CIF (continuous integrate-and-fire) segment-reduce kernel for 8 trn2 NeuronCores.

Math: with alphas rescaled per batch so sum_t alpha = target_len, the scan
    integrate += alpha; fire when integrate >= 0.95; subtract 1 at fire
has the closed form  n_t = floor(csum_t + 0.05)  (number of fires through t),
valid because every alpha < 1.  Each step t contributes to at most two output
tokens:
    c1_t = alpha_t - c2_t           -> token n_{t-1}
    c2_t = fire_t * (csum_t - n_t)  -> token n_t        (fire_t = n_t - n_{t-1})
Contributions to tokens >= target_len are dropped (trailing partial frame).
So   out[b] = C_b[L, T] @ hidden[b][T, H]   with a banded C built on-device
from the cumsum, and the matmul done on the PE in fp16 (1 cycle/row; fp32r
runs at ~2) with fp32 PSUM accumulation.

The cumsum itself is computed on the host in float64 (64K adds — descriptor-
style preprocessing; fp32 accumulation at magnitude ~256 would jitter fire
positions by ~1e-4 and flip occasional fire steps vs the reference).

Sharding: pure data parallel, 4 batches per core.
"""

import sys

if "/opt/trn_rl_repo" not in sys.path:
    sys.path.insert(0, "/opt/trn_rl_repo")

import numpy as np

import concourse.bass as bass
import concourse.bacc as bacc
import concourse.tile as tile
from concourse import mybir
from concourse import bass_utils

F32 = mybir.dt.float32
F32R = mybir.dt.float32r
F16 = mybir.dt.float16
I32 = mybir.dt.int32
A = mybir.AluOpType

# Problem constants (hardcoded per the harness contract).
B, T, H, L = 32, 2000, 512, 256
NCORES = 8
BL = B // NCORES  # 4 batches per core
THRESH_OFF = 1.0 - 0.95  # fire threshold offset: n_t = floor(csum + 0.05)


def build_nc(bl=BL, t=T, h=H, l=L, n_cores=NCORES):
    """Build the bass program for one core (SPMD across n_cores)."""
    tp = ((t + 127) // 128) * 128  # padded time
    nt = tp // 128                 # number of T-tiles
    ncol = bl * nt                 # columns in the [128, ncol] "column layout"

    nc = bacc.Bacc(
        "TRN2",
        target_bir_lowering=False,
        debug=False,
        num_devices=n_cores,
    )

    # h is host-interleaved to [t, bl*h]: row t holds all bl batches' step-t
    # vectors back to back -> fat contiguous DMA packets per SBUF partition.
    # fp16: halves DMA bytes and runs the PE at 1 cycle/row (fp32r is ~2).
    h_d = nc.dram_tensor("h", [t, bl * h], F16, kind="ExternalInput").ap()
    # csumz[b, 0] = 0; csumz[b, 1+j] = cumsum(rescaled alphas)[j] (flat pad)
    cz_d = nc.dram_tensor("csumz", [bl, tp + 1], F32, kind="ExternalInput").ap()
    tgtbc_d = nc.dram_tensor("tgt_bc", [128, ncol], F32, kind="ExternalInput").ap()
    iota_d = nc.dram_tensor("iota", [128, l], F16, kind="ExternalInput").ap()
    ident_d = nc.dram_tensor("ident", [bl, bl], F32, kind="ExternalInput").ap()
    # out is interleaved the same way: [l, bl*h]; host de-interleaves.
    out_d = nc.dram_tensor("out", [l, bl * h], F32, kind="ExternalOutput").ap()

    with tile.TileContext(nc) as tc:
        _body(tc, nc, h_d, cz_d, tgtbc_d, iota_d, ident_d, out_d,
              bl=bl, t=t, h=h, l=l, tp=tp, nt=nt, ncol=ncol)

    nc.compile()
    return nc


def _body(tc, nc, h_d, cz_d, tgtbc_d, iota_d, ident_d, out_d,
          *, bl, t, h, l, tp, nt, ncol):
    with (
        tc.tile_pool(name="const", bufs=1) as cpool,
        tc.tile_pool(name="prep", bufs=1) as prep,
        tc.tile_pool(name="pcol", bufs=1) as pcol,
    ):
        # ---- constants from host ----
        iota_sb = cpool.tile([128, l], F16)
        nc.sync.dma_start(iota_sb[:], iota_d[:])
        ident_sb = cpool.tile([bl, bl], F32)
        nc.sync.dma_start(ident_sb[:], ident_d[:])
        tgtbc_sb = cpool.tile([128, ncol], F32)
        nc.sync.dma_start(tgtbc_sb[:], tgtbc_d[:])

        cz_row = prep.tile([bl, tp + 1], F32)
        nc.sync.dma_start(cz_row[:], cz_d[:])

        # ---- transpose csum / csum_prev into column layout [128, ncol] ----
        # column (i*bl + b) of cs_all = csum[b, 128*i + p] = cz_row[b, 1+128i+p]
        cs_all = pcol.tile([128, ncol], F32)
        csp_all = pcol.tile([128, ncol], F32)
        with tc.tile_pool(name="tpp", bufs=4, space="PSUM") as tpp:
            for i in range(nt):
                pt = tpp.tile([128, bl], F32, tag="pt")
                nc.tensor.transpose(
                    pt[:], cz_row[0:bl, 1 + i * 128:1 + (i + 1) * 128],
                    ident_sb[:])
                nc.vector.tensor_copy(cs_all[:, i * bl:(i + 1) * bl], pt[:])
                pt2 = tpp.tile([128, bl], F32, tag="pt")
                nc.tensor.transpose(
                    pt2[:], cz_row[0:bl, i * 128:(i + 1) * 128], ident_sb[:])
                nc.vector.tensor_copy(csp_all[:, i * bl:(i + 1) * bl], pt2[:])

        # ---- column-layout math: coefficients ----
        def floor_col(dst, src):
            # dst = floor(src + THRESH_OFF), exact for 0 <= src < 2^23
            x = pcol.tile([128, ncol], F32, tag="fl_x")
            xi = pcol.tile([128, ncol], I32, tag="fl_i")
            xf = pcol.tile([128, ncol], F32, tag="fl_f")
            nc.vector.tensor_scalar(x[:], src[:], THRESH_OFF, None, A.add)
            nc.vector.tensor_copy(xi[:], x[:])   # f32 -> i32 (round or trunc)
            nc.vector.tensor_copy(xf[:], xi[:])  # i32 -> f32 (exact)
            nc.vector.tensor_tensor(dst[:], xf[:], x[:], A.is_gt)
            nc.vector.tensor_tensor(dst[:], xf[:], dst[:], A.subtract)

        al_all = pcol.tile([128, ncol], F32)
        nc.vector.tensor_tensor(al_all[:], cs_all[:], csp_all[:], A.subtract)

        n_all = pcol.tile([128, ncol], F32)
        floor_col(n_all, cs_all)
        np_all = pcol.tile([128, ncol], F32)
        floor_col(np_all, csp_all)

        fire = pcol.tile([128, ncol], F32)
        nc.vector.tensor_tensor(fire[:], n_all[:], np_all[:], A.subtract)
        d_all = pcol.tile([128, ncol], F32)
        nc.vector.tensor_tensor(d_all[:], cs_all[:], n_all[:], A.subtract)
        c2 = pcol.tile([128, ncol], F32)
        nc.vector.tensor_tensor(c2[:], fire[:], d_all[:], A.mult)
        c1 = pcol.tile([128, ncol], F32)
        nc.vector.tensor_tensor(c1[:], al_all[:], c2[:], A.subtract)

        # validity: contributions to token >= target are dropped
        v_t = pcol.tile([128, ncol], F32, tag="v_t")
        nc.vector.tensor_tensor(v_t[:], np_all[:], tgtbc_sb[:], A.is_lt)
        nc.vector.tensor_tensor(c1[:], c1[:], v_t[:], A.mult)
        v_t2 = pcol.tile([128, ncol], F32, tag="v_t2")
        nc.vector.tensor_tensor(v_t2[:], n_all[:], tgtbc_sb[:], A.is_lt)
        nc.vector.tensor_tensor(c2[:], c2[:], v_t2[:], A.mult)

        # ---- PE warm-up: dummy matmuls gated on the transposed csum so
        # they overlap the coefficient math and the HAM clock gate is at
        # 2.4 GHz when the real matmul stream starts ----
        with tc.tile_pool(name="warm", bufs=1, space="PSUM") as wp:
            wps = wp.tile([128, l], F32)
            wm = min(ncol, 128)
            wn = min(l, ncol)
            for _ in range(8):
                nc.tensor.matmul(
                    wps[0:wm, 0:wn], cs_all[:, 0:wm], csp_all[:, 0:wn],
                    start=True, stop=True)

        # ---- main loop: build C^T tiles, matmul, accumulate ----
        nl = l // 128  # number of 128-row output chunks (2)
        with (
            tc.tile_pool(name="hin", bufs=6) as hp,
            tc.tile_pool(name="ct", bufs=6) as ctp,
            tc.tile_pool(name="acc", bufs=1, space="PSUM") as accp,
            tc.tile_pool(name="osb", bufs=4) as osb,
        ):
            psums = [
                [accp.tile([128, h], F32, tag=f"ps{b}{c}", name=f"ps{b}{c}")
                 for c in range(nl)]
                for b in range(bl)
            ]
            for i in range(nt):
                h_t = hp.tile([128, bl * h], F16, tag="h_t")
                tlo, thi = i * 128, min(t, (i + 1) * 128)
                if thi - tlo < 128:
                    nc.gpsimd.memset(h_t[64:128, :], 0.0)
                nc.sync.dma_start(h_t[0:thi - tlo, :], h_d[tlo:thi, :])

                for b in range(bl):
                    col = i * bl + b
                    # 3-op DVE chain: zero-padded one-hot mask of tok1,
                    # scale by c1, then one scalar_tensor_tensor reads the
                    # mask shifted one column left to add the c2 term at
                    # token tok1+1 (tok2 = tok1+1 always; c2 = 0 on no-fire
                    # steps, and a tok2 of 256 falls off the tile edge).
                    ct = ctp.tile([128, l], F16, tag="ct")
                    mask = ctp.tile([128, l + 2], F16, tag="mask")
                    cta = ctp.tile([128, l], F16, tag="cta")
                    nc.gpsimd.memset(mask[:, 0:2], 0.0)
                    nc.vector.tensor_scalar(
                        mask[:, 2:l + 2], iota_sb[:],
                        np_all[:, col:col + 1], None, A.is_equal)
                    nc.vector.tensor_scalar(
                        cta[:], mask[:, 2:l + 2], c1[:, col:col + 1],
                        None, A.mult)
                    nc.vector.scalar_tensor_tensor(
                        ct[:], mask[:, 1:l + 1], c2[:, col:col + 1],
                        cta[:], A.mult, A.add)

                    for c in range(nl):
                        nc.tensor.matmul(
                            psums[b][c][:],
                            ct[:, c * 128:(c + 1) * 128],
                            h_t[:, b * h:(b + 1) * h],
                            start=(i == 0), stop=(i == nt - 1))

            for c in range(nl):
                ot = osb.tile([128, bl * h], F32, tag="ot")
                for b in range(bl):
                    # split the tail evacuations across ACT and DVE
                    eng = nc.scalar.copy if b % 2 == 0 else nc.vector.tensor_copy
                    eng(ot[:, b * h:(b + 1) * h], psums[b][c][:])
                nc.sync.dma_start(out_d[c * 128:(c + 1) * 128, :], ot[:])


_cached_nc = None


def _get_nc():
    global _cached_nc
    if _cached_nc is None:
        _cached_nc = build_nc()
    return _cached_nc


def make_in_maps(hidden, alphas, target_lengths, bl=BL, t=T, l=L, n_cores=NCORES):
    tp = ((t + 127) // 128) * 128
    nt = tp // 128
    hidden = np.ascontiguousarray(np.asarray(hidden, dtype=np.float32))
    tl = np.asarray(target_lengths)

    # host-side f64 cumsum of rescaled alphas (exact-arithmetic fire positions)
    a64 = np.asarray(alphas, dtype=np.float64)
    scale = tl.astype(np.float64) / a64.sum(axis=1)
    csum = np.cumsum(a64 * scale[:, None], axis=1)  # [B, t]
    csumz = np.zeros((a64.shape[0], tp + 1), np.float64)
    csumz[:, 1:1 + t] = csum
    csumz[:, 1 + t:] = csum[:, -1:]
    csumz = csumz.astype(np.float32)

    iota = np.ascontiguousarray(
        np.broadcast_to(np.arange(l, dtype=np.float16)[None, :], (128, l)))
    ident = np.eye(bl, dtype=np.float32)
    in_maps = []
    for c in range(n_cores):
        sl = slice(c * bl, (c + 1) * bl)
        tgt = tl[sl].astype(np.float32)
        # interleave batches along the feature axis: h_r[t, b*h:(b+1)*h]
        h_r = np.ascontiguousarray(
            hidden[sl].transpose(1, 0, 2).reshape(t, bl * hidden.shape[2])
            .astype(np.float16))
        in_maps.append({
            "h": h_r,
            "csumz": np.ascontiguousarray(csumz[sl]),
            "tgt_bc": np.ascontiguousarray(
                np.broadcast_to(np.tile(tgt, nt)[None, :], (128, bl * nt))),
            "iota": iota,
            "ident": ident,
        })
    return in_maps


def kernel(hidden, alphas, target_lengths):
    nc = _get_nc()
    in_maps = make_in_maps(hidden, alphas, target_lengths)
    res = bass_utils.run_bass_kernel_spmd(
        nc, in_maps, core_ids=list(range(NCORES)))
    return assemble_out(res.results)


def assemble_out(results):
    # de-interleave [l, bl*h] -> [bl, l, h] per core
    outs = [r["out"].reshape(L, BL, H).transpose(1, 0, 2) for r in results]
    return np.ascontiguousarray(np.concatenate(outs, axis=0))


if __name__ == "__main__":
    # smoke test with random inputs
    rng = np.random.default_rng(0)
    hidden = rng.standard_normal((B, T, H), dtype=np.float32)
    alphas = rng.random((B, T), dtype=np.float32)
    tl = rng.integers(64, L + 1, size=(B,)).astype(np.int64)
    out = kernel(hidden, alphas, tl)
    print("out", out.shape, out.dtype, float(np.abs(out).sum()))
